# revision 1
# baseline (speedup 1.0000x reference)
"""Trainium2 Bass kernel for the CRF forward algorithm (nn_CRF).

Reference computes: scan over S=8192 steps of
    fv'[i] = logsumexp_j(fv[j] + transitions[i, j]) + h[s, i]
then logsumexp(fv + transitions[END_IDX]).

Algorithm used here (exp-space scan with exact running normalizer):
    W = exp(transitions)            (computed on device, bf16)
    v = fv - C   (normalized state, [2048] fp32)
    per step:
        w   = exp(v)                           (bf16)
        E   = W @ w                            (PE matvec, fp32 psum)
        mh  = ln(colsum . w)  = ln(sum_i E_i)  (PE skinny matmul + ACT Ln)
        v'  = ln(E + 1e-30) + h[s] - mh
        C  += mh
    answer = C + ln(sum_i exp(v_i + transitions[END_IDX, i]))
mh = ln(sum_i E_i) >= max_i ln(E_i), and <= max + ln(2048), so max(v) stays
bounded in ~[-13, +6]: exp never overflows and dominant terms never flush.
mh cancels exactly between C and v', so only its magnitude matters.
colsum[j] = sum_i exp(tr[i,j]) is precomputed on device.

Layout: tag j -> (slot k = j//128, partition p = j%128); v/w/h tiles are
[128, 16].  W^T lives in SBUF as 256 bf16 tiles [128 j, 128 i], tile
t = k*16 + g at free offset t*128 (k = j-slot, g = i-group).

Each of the 8 cores runs the identical full recurrence (replicated); core 0's
output is returned.  (Cross-core remote-DMA crashes this runtime and in-loop
collectives are architecturally impossible, so replication is the reliable
distribution.)
"""
import sys

sys.path.insert(0, "/opt/trn_rl_repo")

import numpy as np

S = 8192
T = 2048
P = 128
NSLOT = T // P          # 16 j-slots
NGRP = T // P           # 16 i-groups
NBLK = NSLOT * NGRP     # 256 W tiles
UNROLL = 2              # steps per loop iteration (h double-buffer parity)
EPS = 1e-30


def build_kernel(n_steps=S, hsb_rows=None, timing_mode=False):
    import concourse.bacc as bacc
    import concourse.bass as bass
    import concourse.mybir as mybir
    from contextlib import ExitStack

    if timing_mode:
        hsb_rows = 2
    hsb_rows = hsb_rows or n_steps
    assert n_steps % UNROLL == 0
    fp32 = mybir.dt.float32
    bf16 = mybir.dt.bfloat16
    AF = mybir.ActivationFunctionType
    ALU = mybir.AluOpType
    AX = mybir.AxisListType

    nc = bacc.Bacc("TRN2", target_bir_lowering=True, num_devices=8)

    n_wtb = 2 if timing_mode else NBLK
    wtb = nc.declare_dram_parameter("wtb", [n_wtb, P, P], fp32, isOutput=False)
    hsb = nc.declare_dram_parameter("hsb", [hsb_rows, T], fp32, isOutput=False)
    v0f = nc.declare_dram_parameter("v0f", [P, NSLOT], fp32, isOutput=False)
    trendf = nc.declare_dram_parameter("trendf", [P, NSLOT], fp32, isOutput=False)
    out_d = nc.declare_dram_parameter("out", [1, 1], fp32, isOutput=True)

    ctx = ExitStack()
    sb = lambda name, shape, dt: ctx.enter_context(nc.sbuf_tensor(name, shape, dt))
    ps = lambda name, shape, dt: ctx.enter_context(nc.psum_tensor(name, shape, dt))
    sem = lambda name: ctx.enter_context(nc.semaphore(name))

    with ctx:
        wt = sb("wt", [P, NBLK * P], bf16)   # W^T, 256 tiles of [128,128]
        colsum = sb("colsum", [P, NSLOT], fp32)
        colsum_bf = sb("colsum_bf", [P, NSLOT], bf16)
        v = sb("v", [P, NSLOT], fp32)
        w = sb("w", [P, NSLOT], bf16)
        ln_out = sb("ln_out", [P, NSLOT], fp32)
        es = sb("es", [P, NSLOT], fp32)      # h[s] - mh ; reused at the end
        h_step = [sb(f"h_step{i}", [P, NSLOT], fp32) for i in range(UNROLL)]
        tmp = [sb(f"tmp{i}", [P, P], fp32) for i in range(2)]
        ones_col = sb("ones_col", [P, 1], fp32)
        eps_t = sb("eps_t", [P, 1], fp32)
        ones_row = sb("ones_row", [1, P], fp32)
        m_sb = sb("m_sb", [1, 1], fp32)      # mh scalar
        c_acc = sb("c_acc", [1, 1], fp32)    # C accumulator
        trend = sb("trend", [P, NSLOT], fp32)
        fin = sb("fin", [1, 1], fp32)

        psum_mv = ps("psum_mv", [P, NSLOT], fp32)
        psum_m = ps("psum_m", [1, 1], fp32)
        psum_b = ps("psum_b", [P, 1], fp32)
        psum_f = ps("psum_f", [1, 1], fp32)

        su_dma = [sem("su_dma0"), sem("su_dma1")]  # wtb DMAs by parity
        su_exp = sem("su_exp")       # setup exp done (+1 per block)
        su_misc = sem("su_misc")     # consts / v0 / trend ready
        h_ready = [sem("h_ready0"), sem("h_ready1")]  # h DMA by parity
        w_sem = sem("w_sem")         # ACT exp done (+1 per step)
        pe1 = sem("pe1")             # PE mv+skinny done (+1 per step)
        pe2 = sem("pe2")             # PE mh-bcast done (+1 per step)
        act_ln = sem("act_ln")       # ACT Ln pair done (+1 per step)
        dve_st = sem("dve_st")       # DVE step done (+1 per step)
        fin_sem = sem("fin_sem")

        n_iter = n_steps // UNROLL

        with nc.Block() as block:

            # ---------------- sync engine: all input DMAs ----------------
            @block.sync
            def _(eng):
                eng.dma_start(v[:, :], v0f[:, :]).then_inc(su_misc, 16)
                eng.dma_start(trend[:, :], trendf[:, :]).then_inc(su_misc, 16)
                for t in range(NBLK):
                    if t >= 2:
                        eng.wait_ge(su_exp, t - 1)
                    eng.dma_start(
                        tmp[t % 2][:, :],
                        wtb[(t % 2 if timing_mode else t), :, :],
                    ).then_inc(su_dma[t % 2], 16)
                # h prologue: steps 0..UNROLL-1
                for s in range(UNROLL):
                    eng.dma_start(
                        h_step[s][:, :],
                        hsb[(0 if timing_mode else s) : (1 if timing_mode else s + 1), :],
                    ).then_inc(h_ready[s % 2], 16)
                r_off = eng.alloc_register("r_off")   # step index
                r_g = eng.alloc_register("r_g")       # dve_st guard
                r_i = eng.alloc_register("r_i")
                eng.reg_mov(r_off, 0 if timing_mode else UNROLL)
                eng.reg_mov(r_g, 0)
                eng.reg_mov(r_i, 0)
                eng.br("sync_loop")
                with nc.body("sync_loop"):
                    for u in range(UNROLL):
                        eng.reg_add(r_g, r_g, 1)
                        eng.wait_ge(dve_st, r_g)
                        eng.dma_start(
                            h_step[u][:, :],
                            hsb[bass.ds(eng.snap(r_off), 1), :],
                        ).then_inc(h_ready[u % 2], 16)
                        if not timing_mode:
                            eng.reg_add(r_off, r_off, 1)
                    eng.reg_add(r_i, r_i, 1)
                    eng.br_lt(r_i, n_iter - 1, "sync_loop", "sync_done")
                with nc.body("sync_done"):
                    eng.wait_ge(fin_sem, 5)
                    eng.dma_start(out_d[:, :], fin[:, :]).then_inc(su_misc, 16)
                    eng.br(block.end_bb)

            # ---------------- gpsimd: constants only ----------------
            @block.gpsimd
            def _(eng):
                eng.memset(ones_col[:, :], 1.0)
                eng.memset(eps_t[:, :], EPS)
                eng.memset(ones_row[:, :], 1.0)
                eng.memset(c_acc[:, :], 0.0)
                eng.drain()
                eng.nop().then_inc(su_misc, 16)

            # ------------- scalar (ACT): W exp setup, loop exp/ln ----------
            @block.scalar
            def _(eng):
                for t in range(NBLK):
                    eng.wait_ge(su_dma[t % 2], 16 * (t // 2 + 1))
                    eng.activation(
                        wt[:, t * P : (t + 1) * P], tmp[t % 2][:, :], AF.Exp
                    ).then_inc(su_exp, 1)
                r_v = eng.alloc_register("r_v")    # dve_st target
                r_pe = eng.alloc_register("r_pe")  # pe1 target
                r_i = eng.alloc_register("r_i")
                eng.reg_mov(r_v, 0)
                eng.reg_mov(r_pe, 0)
                eng.reg_mov(r_i, 0)
                eng.wait_ge(su_misc, 48)
                eng.br("act_loop")
                with nc.body("act_loop"):
                    for u in range(UNROLL):
                        eng.wait_ge(dve_st, r_v)      # v from prev step
                        eng.wait_ge(pe1, r_pe)        # w free (prev matvec)
                        eng.activation(w[:, :], v[:, :], AF.Exp).then_inc(
                            w_sem, 1
                        )
                        eng.reg_add(r_pe, r_pe, 1)
                        eng.wait_ge(pe1, r_pe)        # this step's matvec done
                        eng.activation(
                            ln_out[:, :], psum_mv[:, :], AF.Ln,
                            bias=eps_t[:, :],
                        )
                        eng.activation(m_sb[:, :], psum_m[:, :], AF.Ln).then_inc(
                            act_ln, 1
                        )
                        eng.reg_add(r_v, r_v, 1)
                    eng.reg_add(r_i, r_i, 1)
                    eng.br_lt(r_i, n_iter, "act_loop", "act_fin")
                with nc.body("act_fin"):
                    eng.wait_ge(fin_sem, 1)
                    eng.activation(ln_out[:, :], es[:, :], AF.Exp).then_inc(
                        fin_sem, 1
                    )
                    eng.wait_ge(pe2, n_steps + 1)
                    eng.activation(m_sb[:, :], psum_f[:, :], AF.Ln).then_inc(
                        fin_sem, 1
                    )
                    eng.br(block.end_bb)

            # ------------- tensor (PE): matvec + skinny + bcast -------------
            @block.tensor
            def _(eng):
                r_w = eng.alloc_register("r_w")
                r_ln = eng.alloc_register("r_ln")
                r_dve = eng.alloc_register("r_dve")
                r_i = eng.alloc_register("r_i")
                eng.reg_mov(r_w, 0)
                eng.reg_mov(r_ln, 0)
                eng.reg_mov(r_dve, 0)
                eng.reg_mov(r_i, 0)
                eng.wait_ge(su_misc, 64)
                eng.br("pe_loop")
                with nc.body("pe_loop"):
                    for u in range(UNROLL):
                        eng.reg_add(r_w, r_w, 1)
                        eng.wait_ge(w_sem, r_w)       # w ready
                        eng.wait_ge(act_ln, r_ln)     # psum_mv/m free
                        eng.wait_ge(dve_st, r_dve)    # psum_b free
                        for g in range(NGRP):
                            for k in range(NSLOT):
                                t = k * NGRP + g
                                eng.matmul(
                                    psum_mv[:, g : g + 1],
                                    wt[:, t * P : (t + 1) * P],
                                    w[:, k : k + 1],
                                    start=(k == 0),
                                    stop=(k == NSLOT - 1),
                                )
                        for k in range(NSLOT):
                            mm = eng.matmul(
                                psum_m[:, :],
                                colsum_bf[:, k : k + 1],
                                w[:, k : k + 1],
                                start=(k == 0),
                                stop=(k == NSLOT - 1),
                            )
                            if k == NSLOT - 1:
                                mm.then_inc(pe1, 1)
                        eng.reg_add(r_ln, r_ln, 1)
                        eng.wait_ge(act_ln, r_ln)     # mh ready
                        eng.matmul(
                            psum_b[:, :],
                            ones_row[:, :],
                            m_sb[:, :],
                            start=True,
                            stop=True,
                        ).then_inc(pe2, 1)
                        eng.reg_add(r_dve, r_dve, 1)
                    eng.reg_add(r_i, r_i, 1)
                    eng.br_lt(r_i, n_iter, "pe_loop", "pe_fin")
                with nc.body("pe_fin"):
                    eng.wait_ge(fin_sem, 3)
                    eng.matmul(
                        psum_f[:, :],
                        es[:, 0:1],
                        ones_col[:, :],
                        start=True,
                        stop=True,
                    ).then_inc(pe2, 1)
                    eng.br(block.end_bb)

            # ------------- vector (DVE): colsum setup + per-step tail -------
            @block.vector
            def _(eng):
                for k in range(NSLOT):
                    eng.wait_ge(su_exp, (k + 1) * NGRP)
                    eng.tensor_reduce(
                        colsum[:, k : k + 1],
                        wt[:, k * NGRP * P : (k + 1) * NGRP * P],
                        axis=AX.X,
                        op=ALU.add,
                    )
                eng.drain()
                eng.tensor_copy(colsum_bf[:, :], colsum[:, :]).then_inc(
                    su_misc, 16
                )
                r_pe2 = eng.alloc_register("r_pe2")
                r_ln = eng.alloc_register("r_ln")
                r_h = eng.alloc_register("r_h")
                r_wr = eng.alloc_register("r_wr")
                r_i = eng.alloc_register("r_i")
                eng.reg_mov(r_pe2, 0)
                eng.reg_mov(r_ln, 0)
                eng.reg_mov(r_h, 0)
                eng.reg_mov(r_wr, 0)
                eng.reg_mov(r_i, 0)
                eng.wait_ge(su_misc, 48)
                eng.br("dve_loop")
                with nc.body("dve_loop"):
                    for u in range(UNROLL):
                        eng.reg_add(r_pe2, r_pe2, 1)
                        eng.reg_add(r_ln, r_ln, 1)
                        if u == 0:
                            eng.reg_add(r_h, r_h, 16)
                        eng.reg_add(r_wr, r_wr, 1)
                        eng.wait_ge(h_ready[u % 2], r_h)
                        eng.wait_ge(pe2, r_pe2)       # psum_b (mh bcast)
                        eng.drain()                   # es WAR vs prev v-add
                        eng.tensor_scalar(
                            es[:, :],
                            h_step[u][:, :],
                            psum_b[:, :],
                            None,
                            op0=ALU.subtract,
                        )
                        eng.tensor_tensor(
                            c_acc[:, :], c_acc[:, :], m_sb[:, :], op=ALU.add
                        )
                        eng.drain()                   # es RAW
                        eng.wait_ge(act_ln, r_ln)     # ln_out ready
                        eng.wait_ge(w_sem, r_wr)      # exp done reading v
                        eng.tensor_tensor(
                            v[:, :], ln_out[:, :], es[:, :], op=ALU.add
                        ).then_inc(dve_st, 1)
                    eng.reg_add(r_i, r_i, 1)
                    eng.br_lt(r_i, n_iter, "dve_loop", "dve_fin")
                with nc.body("dve_fin"):
                    eng.drain()
                    eng.tensor_tensor(
                        es[:, :], v[:, :], trend[:, :], op=ALU.add
                    ).then_inc(fin_sem, 1)
                    eng.wait_ge(fin_sem, 2)           # ACT exp(es) done
                    eng.drain()
                    eng.tensor_reduce(
                        es[:, 0:1], ln_out[:, :], axis=AX.X, op=ALU.add
                    ).then_inc(fin_sem, 1)
                    eng.wait_ge(fin_sem, 4)           # ACT Ln(psum_f) -> m_sb
                    eng.drain()
                    eng.tensor_tensor(
                        fin[:, :], m_sb[:, :], c_acc[:, :], op=ALU.add
                    ).then_inc(fin_sem, 1)
                    eng.br(block.end_bb)

    nc.compile()
    return nc


_NC_CACHE = {}


def _get_nc(n_steps):
    if n_steps not in _NC_CACHE:
        _NC_CACHE[n_steps] = build_kernel(n_steps)
    return _NC_CACHE[n_steps]


def prep_inputs(h, transitions):
    h = np.ascontiguousarray(np.asarray(h, dtype=np.float32))
    tr = np.ascontiguousarray(np.asarray(transitions, dtype=np.float32))
    n_steps = h.shape[0]
    # p-major tag layout: tag j <-> (p = j // NSLOT, k = j % NSLOT)
    wtb = np.empty((NBLK, P, P), dtype=np.float32)
    for k in range(NSLOT):
        for g in range(NGRP):
            wtb[k * NGRP + g] = tr[g::NGRP, :][:, k::NSLOT].T
    v0 = np.full((T,), -10000.0, dtype=np.float32)
    v0[0] = 0.0
    return {
        "wtb": np.ascontiguousarray(wtb),
        "hsb": h,
        "v0f": np.ascontiguousarray(v0.reshape(P, NSLOT)),
        "trendf": np.ascontiguousarray(tr[1].reshape(P, NSLOT)),
    }


def kernel(h, transitions):
    from concourse.bass_utils import run_bass_kernel_spmd

    inputs = prep_inputs(h, transitions)
    n_steps = inputs["hsb"].shape[0]
    nc = _get_nc(n_steps)
    core_ids = list(range(8))
    in_maps = [dict(inputs) for _ in core_ids]
    res = run_bass_kernel_spmd(nc, in_maps, core_ids)
    return np.asarray(res.results[0]["out"][0, 0], dtype=np.float32)


if __name__ == "__main__":
    import reference

    inputs = {k: np.asarray(v) for k, v in reference.setup_inputs().items()}
    out = kernel(**inputs)
    print("kernel out:", out)



# revision 13
# speedup vs baseline: 1698.3611x; 1698.3611x over previous
"""Trainium2 Bass kernel for the CRF forward algorithm (nn_CRF).

Reference: scan over S=8192 steps of
    fv'[i] = logsumexp_j(fv[j] + transitions[i, j]) + h[s, i]
then logsumexp(fv + transitions[END_IDX]).

Distribution (no cross-core communication): the step maps are products of
strictly positive matrices, so the normalized forward state forgets its
init at ~0.04x/step (measured: < 1e-13 deviation by step 12).  The 8192
steps split into 8 chunks stitched only by scalars: core c scans h rows
[c*1020, c*1020 + 1052); 32 burn-in steps from a zeros init (core 0: the
true CRF init), captures the state at local step 32 ("y" = global state
at the previous core's end cut) and at local step 1052 ("X" = its own end
cut).  Each core's y has the same direction as the previous core's X, so
the unknown additive constants follow by matching LSEs on the host, which
also runs the terminal logsumexp in float64.

Device algorithm (exp-space state, lazy normalization every 4 steps
applied with a 1-iteration lag as an exp bias; critical path per step is
just PE mv-block -> DVE multiply -> PE mv-block):
    E    = W @ w                      (PE, 256 LDW+MM pairs, fp32 psum)
    w'   = E * exp(h[s] + bias)       (ACT precomputes eh; DVE multiplies)
    bias = -ln(sigma E) from the previous iteration's skinny matmul,
           applied only at u=0; sigma E measured at u=3 (16 skinny MMs);
           C += ln(sigma E) bookkept on DVE; ACT computes the Ln.
Captures are (ln(w + eps), C); the invariant ln(w) + C is exact at
iteration boundaries regardless of when normalization is applied.

Layout: tag j -> (p = j // 16, slot k = j % 16); w/eh/h tiles [128, 16].
W^T in SBUF as 256 bf16 tiles [128 j, 128 i], tile t = k*16 + g.
"""
import sys

sys.path.insert(0, "/opt/trn_rl_repo")

import numpy as np

S = 8192
T = 2048
P = 128
NSLOT = T // P
NGRP = T // P
NBLK = NSLOT * NGRP
UNROLL = 4
EPS = 1e-30

NSTEPS = 1052           # 8*NSTEPS - 7*BURN == S
BURN = 32
STRIDE = NSTEPS - BURN  # 1020


def build_kernel(n_steps=NSTEPS, timing_mode=False, pe_only=False):
    import concourse.bacc as bacc
    import concourse.bass as bass
    import concourse.mybir as mybir
    from contextlib import ExitStack

    hsb_rows = 2 if timing_mode else n_steps
    assert n_steps % UNROLL == 0 and n_steps >= BURN + UNROLL
    n_iter = n_steps // UNROLL
    n_iter_a = BURN // UNROLL      # 8
    n_iter_b = n_iter - n_iter_a
    fp32 = mybir.dt.float32
    bf16 = mybir.dt.bfloat16
    AF = mybir.ActivationFunctionType
    ALU = mybir.AluOpType
    AX = mybir.AxisListType

    nc = bacc.Bacc("TRN2", target_bir_lowering=True, num_devices=8)

    n_wtb = 2 if timing_mode else NBLK
    wtb = nc.declare_dram_parameter("wtb", [n_wtb, P, P], fp32, isOutput=False)
    hsb = nc.declare_dram_parameter("hsb", [hsb_rows, T], fp32, isOutput=False)
    v0f = nc.declare_dram_parameter("v0f", [P, NSLOT], fp32, isOutput=False)
    out_y = nc.declare_dram_parameter("out_y", [P, NSLOT], fp32, isOutput=True)
    out_v = nc.declare_dram_parameter("out_v", [P, NSLOT], fp32, isOutput=True)
    out_s = nc.declare_dram_parameter("out_s", [1, 2], fp32, isOutput=True)

    ctx = ExitStack()
    sb = lambda name, shape, dt: ctx.enter_context(nc.sbuf_tensor(name, shape, dt))
    ps = lambda name, shape, dt: ctx.enter_context(nc.psum_tensor(name, shape, dt))
    sem = lambda name: ctx.enter_context(nc.semaphore(name))

    with ctx:
        wt = sb("wt", [P, NBLK * P], bf16)
        colsum = sb("colsum", [P, NSLOT], fp32)
        colsum_bf = sb("colsum_bf", [P, NSLOT], bf16)
        w2 = [sb("w_a", [P, NSLOT], bf16), sb("w_b", [P, NSLOT], bf16)]
        eh = [sb(f"eh{u}", [P, NSLOT], bf16) for u in range(UNROLL)]
        h_step = [sb(f"h_step{u}", [P, NSLOT], fp32) for u in range(UNROLL)]
        v0sb = sb("v0sb", [P, NSLOT], fp32)
        tmp = [sb(f"tmp{i}", [P, P], fp32) for i in range(2)]
        eps_t = sb("eps_t", [P, 1], fp32)
        neg_row = sb("neg_row", [1, P], fp32)
        nmh_sb = sb("nmh_sb", [P, 1], fp32)
        m_sb = sb("m_sb", [1, 1], fp32)
        c_acc = sb("c_acc", [1, 1], fp32)
        ysnap = sb("ysnap", [P, NSLOT], fp32)
        vend = sb("vend", [P, NSLOT], fp32)
        sc_out = sb("sc_out", [1, 2], fp32)

        psum_mv = ps("psum_mv", [P, NSLOT], fp32)
        psum_m = ps("psum_m", [1, 1], fp32)
        psum_b = ps("psum_b", [P, 1], fp32)

        su_dma = [sem("su_dma0"), sem("su_dma1")]
        su_exp = sem("su_exp")
        su_misc = sem("su_misc")
        v0_sem = sem("v0_sem")
        h_ready = [sem(f"h_ready{u}") for u in range(UNROLL)]
        act_eh = sem("act_eh")     # +1 per eh exp (per step)
        act_ln = sem("act_ln")     # +1 per iteration (m_sb valid)
        pe1 = sem("pe1")           # +1 per step (mv block done)
        pe_sig = sem("pe_sig")     # +1 per iteration (skinny done)
        pe2 = sem("pe2")           # +1 per iteration (bcast done)
        dve_st = sem("dve_st")     # +1 per step (w written, psum consumed)
        cap_sem = sem("cap_sem")
        fin_sem = sem("fin_sem")

        with nc.Block() as block:

            # ---------------- sync: input DMAs ----------------
            @block.sync
            def _(eng):
                eng.dma_start(v0sb[:, :], v0f[:, :]).then_inc(v0_sem, 16)
                for t in range(NBLK):
                    if t >= 2:
                        eng.wait_ge(su_exp, t - 1)
                    eng.dma_start(
                        tmp[t % 2][:, :],
                        wtb[(t % 2 if timing_mode else t), :, :],
                    ).then_inc(su_dma[t % 2], 16)
                for u in range(UNROLL):
                    eng.dma_start(
                        h_step[u][:, :],
                        hsb[(0 if timing_mode else u) : (1 if timing_mode else u + 1), :],
                    ).then_inc(h_ready[u], 16)
                r_off = eng.alloc_register("r_off")
                r_g = eng.alloc_register("r_g")
                r_i = eng.alloc_register("r_i")
                eng.reg_mov(r_off, 0 if timing_mode else UNROLL)
                eng.reg_mov(r_g, 0)
                eng.reg_mov(r_i, 0)
                eng.br("sync_done" if pe_only else "sync_loop")
                with nc.body("sync_loop"):
                    for u in range(UNROLL):
                        eng.reg_add(r_g, r_g, 1)
                        eng.wait_ge(act_eh, r_g)
                        eng.dma_start(
                            h_step[u][:, :],
                            hsb[bass.ds(eng.snap(r_off), 1), :],
                        ).then_inc(h_ready[u], 16)
                        if not timing_mode:
                            eng.reg_add(r_off, r_off, 1)
                    eng.reg_add(r_i, r_i, 1)
                    eng.br_lt(r_i, n_iter - 1, "sync_loop", "sync_done")
                with nc.body("sync_done"):
                    eng.wait_ge(fin_sem, 2)
                    eng.dma_start(out_y[:, :], ysnap[:, :]).then_inc(su_misc, 16)
                    eng.dma_start(out_v[:, :], vend[:, :]).then_inc(su_misc, 16)
                    eng.dma_start(out_s[:, :], sc_out[:, :]).then_inc(su_misc, 16)
                    eng.br(block.end_bb)

            # ---------------- gpsimd: constants ----------------
            @block.gpsimd
            def _(eng):
                eng.memset(eps_t[:, :], EPS)
                eng.memset(neg_row[:, :], -1.0)
                eng.memset(c_acc[:, :], 0.0)
                eng.memset(m_sb[:, :], 0.0)
                eng.drain()
                eng.nop().then_inc(su_misc, 16)

            # ---------------- scalar (ACT) ----------------
            @block.scalar
            def _(eng):
                for t in range(NBLK):
                    eng.wait_ge(su_dma[t % 2], 16 * (t // 2 + 1))
                    eng.activation(
                        wt[:, t * P : (t + 1) * P], tmp[t % 2][:, :], AF.Exp
                    ).then_inc(su_exp, 1)
                eng.wait_ge(v0_sem, 16)
                eng.wait_ge(su_misc, 32)
                eng.activation(w2[0][:, :], v0sb[:, :], AF.Exp).then_inc(
                    su_misc, 16
                )
                if pe_only:
                    eng.br(block.end_bb)
                    return
                r_h = eng.alloc_register("r_h")      # h_ready target
                r_p2 = eng.alloc_register("r_p2")    # pe2 target
                r_war = eng.alloc_register("r_war")  # dve_st (eh WAR)
                r_sig = eng.alloc_register("r_sig")  # pe_sig target
                r_dl = eng.alloc_register("r_dl")    # dve_st (m_sb WAR)
                r_it = eng.alloc_register("r_it")
                # peeled iteration 0
                eng.wait_ge(h_ready[0], 16)
                eng.wait_ge(pe2, 1)
                eng.activation(nmh_sb[:, :], psum_b[:, :], AF.Copy)
                eng.drain()
                eng.activation(
                    eh[0][:, :], h_step[0][:, :], AF.Exp, bias=nmh_sb[:, :]
                ).then_inc(act_eh, 1)
                for u in range(1, UNROLL):
                    eng.wait_ge(h_ready[u], 16)
                    eng.activation(
                        eh[u][:, :], h_step[u][:, :], AF.Exp
                    ).then_inc(act_eh, 1)
                eng.wait_ge(pe_sig, 1)
                eng.wait_ge(dve_st, 1)
                eng.activation(m_sb[:, :], psum_m[:, :], AF.Ln).then_inc(
                    act_ln, 1
                )
                eng.reg_mov(r_h, 16)
                eng.reg_mov(r_p2, 1)
                eng.reg_mov(r_war, 0)
                eng.reg_mov(r_sig, 1)
                eng.reg_mov(r_dl, 1)
                eng.reg_mov(r_it, 0)

                def act_iter():
                    eng.reg_add(r_h, r_h, 16)
                    eng.reg_add(r_p2, r_p2, 1)
                    for u in range(UNROLL):
                        eng.reg_add(r_war, r_war, 1)
                        eng.wait_ge(dve_st, r_war)
                        eng.wait_ge(h_ready[u], r_h)
                        if u == 0:
                            eng.wait_ge(pe2, r_p2)
                            eng.activation(nmh_sb[:, :], psum_b[:, :], AF.Copy)
                            eng.drain()
                            eng.activation(
                                eh[0][:, :], h_step[0][:, :], AF.Exp,
                                bias=nmh_sb[:, :],
                            ).then_inc(act_eh, 1)
                        else:
                            eng.activation(
                                eh[u][:, :], h_step[u][:, :], AF.Exp
                            ).then_inc(act_eh, 1)
                    eng.reg_add(r_sig, r_sig, 1)
                    eng.reg_add(r_dl, r_dl, 4)
                    eng.wait_ge(pe_sig, r_sig)
                    eng.wait_ge(dve_st, r_dl)
                    eng.activation(m_sb[:, :], psum_m[:, :], AF.Ln).then_inc(
                        act_ln, 1
                    )

                eng.br("act_loop_a")
                with nc.body("act_loop_a"):
                    act_iter()
                    eng.reg_add(r_it, r_it, 1)
                    eng.br_lt(r_it, n_iter_a - 1, "act_loop_a", "act_cap")
                with nc.body("act_cap"):
                    eng.wait_ge(dve_st, BURN)
                    eng.activation(
                        ysnap[:, :], w2[0][:, :], AF.Ln, bias=eps_t[:, :]
                    ).then_inc(cap_sem, 1)
                    eng.reg_mov(r_it, 0)
                    eng.br("act_loop_b")
                with nc.body("act_loop_b"):
                    act_iter()
                    eng.reg_add(r_it, r_it, 1)
                    eng.br_lt(r_it, n_iter_b, "act_loop_b", "act_fin")
                with nc.body("act_fin"):
                    eng.wait_ge(dve_st, n_steps)
                    eng.activation(
                        vend[:, :], w2[0][:, :], AF.Ln, bias=eps_t[:, :]
                    ).then_inc(fin_sem, 1)
                    eng.br(block.end_bb)

            # ---------------- tensor (PE) ----------------
            @block.tensor
            def _(eng):
                eng.wait_ge(su_misc, 48)

                def mv_block(u):
                    wbuf = w2[u % 2]
                    for g in range(NGRP):
                        for k in range(NSLOT):
                            t = k * NGRP + g
                            mm = eng.matmul(
                                psum_mv[:, g : g + 1],
                                wt[:, t * P : (t + 1) * P],
                                wbuf[:, k : k + 1],
                                start=(k == 0),
                                stop=(k == NSLOT - 1),
                            )
                            if g == NGRP - 1 and k == NSLOT - 1:
                                mm.then_inc(pe1, 1)

                def skinny(u):
                    wbuf = w2[u % 2]
                    for k in range(NSLOT):
                        mm = eng.matmul(
                            psum_m[:, :],
                            colsum_bf[:, k : k + 1],
                            wbuf[:, k : k + 1],
                            start=(k == 0),
                            stop=(k == NSLOT - 1),
                        )
                        if k == NSLOT - 1:
                            mm.then_inc(pe_sig, 1)

                # peeled iteration 0
                eng.matmul(
                    psum_b[:, :], neg_row[:, :], m_sb[:, :], start=True,
                    stop=True,
                ).then_inc(pe2, 1)
                for u in range(UNROLL):
                    if u > 0 and not pe_only:
                        eng.wait_ge(dve_st, u)
                    mv_block(u)
                    if u == UNROLL - 1:
                        skinny(u)
                r_ln = eng.alloc_register("r_ln")
                r_aeh = eng.alloc_register("r_aeh")
                r_dve = eng.alloc_register("r_dve")
                r_it = eng.alloc_register("r_it")
                eng.reg_mov(r_ln, 1)
                eng.reg_mov(r_aeh, 1)
                eng.reg_mov(r_dve, 3)
                eng.reg_mov(r_it, 0)
                eng.br("pe_loop")
                with nc.body("pe_loop"):
                    if not pe_only:
                        eng.wait_ge(act_ln, r_ln)
                        eng.wait_ge(act_eh, r_aeh)
                    eng.reg_add(r_ln, r_ln, 1)
                    eng.reg_add(r_aeh, r_aeh, 4)
                    eng.matmul(
                        psum_b[:, :], neg_row[:, :], m_sb[:, :], start=True,
                        stop=True,
                    ).then_inc(pe2, 1)
                    for u in range(UNROLL):
                        eng.reg_add(r_dve, r_dve, 1)
                        if not pe_only:
                            eng.wait_ge(dve_st, r_dve)
                        mv_block(u)
                        if u == UNROLL - 1:
                            skinny(u)
                    eng.reg_add(r_it, r_it, 1)
                    eng.br_lt(r_it, n_iter - 1, "pe_loop", "pe_fin")
                with nc.body("pe_fin"):
                    eng.br(block.end_bb)

            # ---------------- vector (DVE) ----------------
            @block.vector
            def _(eng):
                for k in range(NSLOT):
                    eng.wait_ge(su_exp, (k + 1) * NGRP)
                    eng.tensor_reduce(
                        colsum[:, k : k + 1],
                        wt[:, k * NGRP * P : (k + 1) * NGRP * P],
                        axis=AX.X,
                        op=ALU.add,
                    )
                eng.drain()
                eng.tensor_copy(colsum_bf[:, :], colsum[:, :]).then_inc(
                    su_misc, 16
                )
                eng.wait_ge(su_misc, 32)
                if pe_only:
                    eng.nop().then_inc(fin_sem, 2)
                    eng.br(block.end_bb)
                    return
                r_p1 = eng.alloc_register("r_p1")
                r_aeh = eng.alloc_register("r_aeh")
                r_ln = eng.alloc_register("r_ln")
                r_it = eng.alloc_register("r_it")
                eng.reg_mov(r_p1, 0)
                eng.reg_mov(r_aeh, 0)
                eng.reg_mov(r_ln, 0)
                eng.reg_mov(r_it, 0)

                def dve_iter():
                    for u in range(UNROLL):
                        eng.reg_add(r_p1, r_p1, 1)
                        eng.reg_add(r_aeh, r_aeh, 1)
                        eng.wait_ge(pe1, r_p1)
                        eng.wait_ge(act_eh, r_aeh)
                        if u == 0:
                            eng.wait_ge(act_ln, r_ln)
                            eng.reg_add(r_ln, r_ln, 1)
                            eng.tensor_tensor(
                                c_acc[:, :], c_acc[:, :], m_sb[:, :],
                                op=ALU.add,
                            )
                        eng.tensor_tensor(
                            w2[(u + 1) % 2][:, :], psum_mv[:, :], eh[u][:, :],
                            op=ALU.mult,
                        ).then_inc(dve_st, 1)

                eng.br("dve_loop_a")
                with nc.body("dve_loop_a"):
                    dve_iter()
                    eng.reg_add(r_it, r_it, 1)
                    eng.br_lt(r_it, n_iter_a, "dve_loop_a", "dve_cap")
                with nc.body("dve_cap"):
                    eng.drain()
                    eng.tensor_copy(sc_out[:, 0:1], c_acc[:, :])
                    eng.wait_ge(cap_sem, 1)
                    eng.reg_mov(r_it, 0)
                    eng.br("dve_loop_b")
                with nc.body("dve_loop_b"):
                    dve_iter()
                    eng.reg_add(r_it, r_it, 1)
                    eng.br_lt(r_it, n_iter_b, "dve_loop_b", "dve_fin")
                with nc.body("dve_fin"):
                    eng.drain()
                    eng.tensor_copy(sc_out[:, 1:2], c_acc[:, :])
                    eng.drain()
                    eng.nop().then_inc(fin_sem, 1)
                    eng.br(block.end_bb)

    nc.compile()
    return nc


_NC_CACHE = {}


def _get_nc(n_steps=NSTEPS, timing_mode=False, pe_only=False):
    key = (n_steps, timing_mode, pe_only)
    if key not in _NC_CACHE:
        _NC_CACHE[key] = build_kernel(
            n_steps, timing_mode=timing_mode, pe_only=pe_only
        )
    return _NC_CACHE[key]


def prep_inputs(h, transitions):
    h = np.ascontiguousarray(np.asarray(h, dtype=np.float32))
    tr = np.ascontiguousarray(np.asarray(transitions, dtype=np.float32))
    assert h.shape == (S, T) and tr.shape == (T, T)
    wtb = np.empty((NBLK, P, P), dtype=np.float32)
    for k in range(NSLOT):
        for g in range(NGRP):
            wtb[k * NGRP + g] = tr[g::NGRP, :][:, k::NSLOT].T
    wtb = np.ascontiguousarray(wtb)
    v0_true = np.full((T,), -10000.0, dtype=np.float32)
    v0_true[0] = 0.0
    in_maps = []
    for c in range(8):
        o = c * STRIDE
        v0 = v0_true if c == 0 else np.zeros((T,), dtype=np.float32)
        in_maps.append(
            {
                "wtb": wtb,
                "hsb": np.ascontiguousarray(h[o : o + NSTEPS]),
                "v0f": np.ascontiguousarray(v0.reshape(P, NSLOT)),
            }
        )
    return in_maps


def _lse64(x):
    m = x.max()
    return m + np.log(np.exp(x - m).sum())


def stitch(results, transitions):
    tr_end = np.asarray(transitions, dtype=np.float64)[1]
    kappa = 0.0
    prev = None
    for c in range(8):
        r = results[c]
        v_y = np.asarray(r["out_y"], dtype=np.float64).reshape(T)
        v_end = np.asarray(r["out_v"], dtype=np.float64).reshape(T)
        c_y = float(r["out_s"][0, 0])
        c_end = float(r["out_s"][0, 1])
        if c > 0:
            pv, pc = prev
            kappa += (pc + _lse64(pv)) - (c_y + _lse64(v_y))
        prev = (v_end, c_end)
    v8, c8 = prev
    return np.float32(_lse64(v8 + tr_end) + c8 + kappa)


def kernel(h, transitions):
    from concourse.bass_utils import run_bass_kernel_spmd

    in_maps = prep_inputs(h, transitions)
    nc = _get_nc()
    res = run_bass_kernel_spmd(nc, in_maps, list(range(8)))
    return stitch(res.results, transitions)


if __name__ == "__main__":
    from ref_numpy import get_inputs

    inputs = get_inputs()
    out = kernel(**inputs)
    print("kernel out:", out)
